# revision 1
# baseline (speedup 1.0000x reference)
"""Trainium2 Bass kernel for the NeuralRadiance embedding-lookup MLP.

Contract: kernel(**inputs) takes the FULL inputs from setup_inputs() and
returns the FULL [N, 3] float32 output.

Strategy (data-parallel over 8 NeuronCores, per sharding hint):
  host: spatial-hash index computation + table lookup, pack rows into
        transposed bf16 tiles laid out for 32-aligned PE row strips.
  device (per core, 262144 rows): 3-layer MLP entirely on-chip.
        L1: bf16 matmul  x[19] @ W1 -> PSUM, relu on DVE -> SBUF bf16
        L2: bf16 matmul h1 @ W2 -> PSUM, relu on ACT -> SBUF bf16
        L3: bf16 block-diag matmul h2 @ [W3;W3] -> PSUM, sigmoid on ACT
  Streams 512-row chunks; two chunks (a "pair") share each PSUM tile so
  the PSUM->SBUF activation passes run at full 128-partition width.
"""

import numpy as np
import ml_dtypes

N = 2_097_152
NC = 8
R = N // NC            # rows per core
L = 512                # rows per chunk (matmul free dim)
CHUNKS = R // L        # 512 chunks per core
MACROS = 32            # input DMA macro-tiles per core ([128, 2048] bf16)
GROUPS = 32            # sigmoid groups per core (16 chunks each)
TABLE = 32768
FEAT = 16
H = 64

_cache = {}


def _hash_idx(pos):
    s = (pos * 8.0).astype(np.int32)
    h = (s[:, 0] * np.int32(73856093)) ^ (s[:, 1] * np.int32(19349663)) ^ (
        s[:, 2] * np.int32(83492791))
    return h & np.int32(TABLE - 1)


def _build_program():
    import concourse.bass as bass
    import concourse.bacc as bacc
    import concourse.tile as tile
    from concourse import mybir

    f32 = mybir.dt.float32
    f32r = mybir.dt.float32r
    bf16 = mybir.dt.bfloat16
    Act = mybir.ActivationFunctionType

    nc = bacc.Bacc(None, target_bir_lowering=False)
    xt_d = nc.dram_tensor("xt", [MACROS, 128, 2048], bf16, kind="ExternalInput")
    w1_d = nc.dram_tensor("w1", [128, H], bf16, kind="ExternalInput")
    w2_d = nc.dram_tensor("w2", [128, H], bf16, kind="ExternalInput")
    w3_d = nc.dram_tensor("w3", [128, 32], bf16, kind="ExternalInput")
    out_d = nc.dram_tensor("out", [GROUPS, 4, 6, 2 * L], f32, kind="ExternalOutput")

    with tile.TileContext(nc) as tc:
        with (
            tc.tile_pool(name="wpool", bufs=1) as wpool,
            tc.tile_pool(name="xin", bufs=3) as xin_pool,
            tc.tile_pool(name="h1", bufs=6) as h1_pool,
            tc.tile_pool(name="h2", bufs=6) as h2_pool,
            tc.tile_pool(name="ot", bufs=2) as ot_pool,
            tc.tile_pool(name="pH1", bufs=2, space="PSUM") as pH1_pool,
            tc.tile_pool(name="pH2", bufs=2, space="PSUM") as pH2_pool,
            tc.tile_pool(name="pO", bufs=1, space="PSUM") as pO_pool,
        ):
            w1t = wpool.tile([128, H], bf16)
            nc.sync.dma_start(out=w1t[:], in_=w1_d[:])
            w2t = wpool.tile([128, H], bf16)
            nc.sync.dma_start(out=w2t[:], in_=w2_d[:])
            w3t = wpool.tile([128, 32], bf16)
            nc.sync.dma_start(out=w3t[:], in_=w3_d[:])

            PAIRS = CHUNKS // 2            # 256 pairs; 8 per macro-tile
            xin_t = {}                     # macro -> xin tile
            h1_t, psH2_t, h2_t = {}, {}, {}
            psO_t = {}

            def xslice(pm, e, xin):
                # even chunk (e=0) lives on strips {2,3}, odd on {0,1}:
                # keeps MM1 row-strips disjoint from MM3's (which always
                # occupy r01/r23 at the matching column halves).
                s = (2 + (pm & 1)) if e == 0 else (pm & 1)
                fs = pm // 2
                return s, xin[32 * s:32 * s + 19, fs * L:(fs + 1) * L]

            # Software-pipelined emission: stage-1 runs two pairs ahead of
            # stage-3 so the in-order PE queue never waits on DVE/ACT.
            S2LAG, S3BASE = 3, 12
            for p in range(PAIRS + S3BASE + 1):
                if p < PAIRS:
                    m, pm = p // 8, p % 8
                    if pm == 0:
                        xin = xin_pool.tile([128, 2048], bf16, name=f"xin{m}",
                                            tag="xin")
                        nc.sync.dma_start(out=xin[:], in_=xt_d[m])
                        xin_t[m] = xin
                    xin = xin_t[m]
                    s0, rhs0 = xslice(pm, 0, xin)
                    s1, rhs1 = xslice(pm, 1, xin)
                    psH1 = pH1_pool.tile([128, L], f32, name=f"psH1_{p}",
                                         tag="psH1")
                    nc.tensor.matmul(
                        out=psH1[0:64, :],
                        lhsT=w1t[32 * s0:32 * s0 + 19, :],
                        rhs=rhs0,
                        start=True, stop=True,
                        tile_position=(32 * s0, 0),
                    )
                    nc.tensor.matmul(
                        out=psH1[64:128, :],
                        lhsT=w1t[32 * s1:32 * s1 + 19, :],
                        rhs=rhs1,
                        start=True, stop=True,
                        tile_position=(32 * s1, 64),
                    )
                    h1t = h1_pool.tile([128, L], bf16, name=f"h1t_{p}",
                                       tag="h1t")
                    h1_t[p] = h1t
                    nc.vector.tensor_scalar_max(h1t[:], psH1[:], 0.0)
                if p >= S2LAG and p - S2LAG < PAIRS:
                    pp = p - S2LAG
                    h1t = h1_t.pop(pp)
                    k, half = pp // 2, pp % 2
                    if half == 0:
                        # psH2d spans two pairs ([128, 2*L] = 2 PSUM banks)
                        # so relu2 runs as one wide ACT op.
                        psH2 = pH2_pool.tile([128, 2 * L], f32,
                                             name=f"psH2_{k}", tag="psH2")
                        psH2_t[k] = psH2
                    psH2 = psH2_t[k]
                    hoff = half * L
                    nc.tensor.matmul(
                        out=psH2[0:64, hoff:hoff + L],
                        lhsT=w2t[0:64, :],
                        rhs=h1t[0:64, :],
                        start=True, stop=True,
                        tile_position=(0, 0),
                    )
                    nc.tensor.matmul(
                        out=psH2[64:128, hoff:hoff + L],
                        lhsT=w2t[64:128, :],
                        rhs=h1t[64:128, :],
                        start=True, stop=True,
                        tile_position=(64, 64),
                    )
                    if half == 1:
                        h2t = h2_pool.tile([128, 2 * L], bf16, name=f"h2t_{k}",
                                           tag="h2t")
                        h2_t[k] = h2t
                        nc.scalar.activation(h2t[:], psH2_t.pop(k)[:], Act.Relu)
                if p >= S3BASE and (p - S3BASE) % 8 == 0                         and (p - S3BASE) // 8 < PAIRS // 8:
                    # Deferred layer-3 burst: 8 back-to-back matmuls with
                    # rotating column strips run ~concurrently, so the
                    # all-row K=128 span stalls the pipe once per 8 pairs
                    # instead of sandwiching every pair.
                    g = (p - S3BASE) // 8
                    psO = pO_pool.tile([128, 2 * L], f32, name=f"psO_{g}",
                                       tag="psO")
                    for qs in range(8):
                        pp = 8 * g + qs
                        r, hh = qs % 4, qs // 4
                        k, half = pp // 2, pp % 2
                        h2t = h2_t[k]
                        nc.tensor.matmul(
                            out=psO[32 * r:32 * r + 32, hh * L:hh * L + L],
                            lhsT=w3t[:],
                            rhs=h2t[:, half * L:half * L + L],
                            start=True, stop=True,
                            tile_position=(0, 32 * r),
                        )
                        if half == 1:
                            del h2_t[k]
                    otile = ot_pool.tile([128, 2 * L], f32, name=f"ot_{g}",
                                         tag="ot")
                    nc.scalar.activation(otile[:], psO[:], Act.Sigmoid)
                    for rr in range(4):
                        nc.sync.dma_start(
                            out=out_d[g, rr],
                            in_=otile[32 * rr:32 * rr + 6, :],
                        )
    nc.finalize()
    return nc


def _get_program():
    if "nc" not in _cache:
        _cache["nc"] = _build_program()
    return _cache["nc"]


def _pack_inputs(pos, normal, emb, W1):
    """Host-side: hash + table lookup + bake transposed bf16 tiles."""
    idx = _hash_idx(pos)
    x19 = np.empty((N, 19), np.float32)
    x19[:, :FEAT] = emb[idx]
    x19[:, FEAT:] = normal
    xv = x19.astype(ml_dtypes.bfloat16)
    # row = ((core*MACROS + m)*16 + c16)*L + j ; c16 = 2*pm + e
    # even chunk (e=0) -> strip 2+(pm&1), odd -> strip (pm&1); slice pm//2
    r = xv.reshape(NC, MACROS, 16, L, 19)
    xt = np.zeros((NC, MACROS, 4, 32, 4, L), ml_dtypes.bfloat16)
    for c16 in range(16):
        pm, e = divmod(c16, 2)
        s = (2 + (pm & 1)) if e == 0 else (pm & 1)
        fs = pm // 2
        xt[:, :, s, :19, fs, :] = r[:, :, c16].transpose(0, 1, 3, 2)
    return xt.reshape(NC, MACROS, 128, 2048)


def _bake_weights(W1, W2, W3):
    w1 = np.zeros((128, H), ml_dtypes.bfloat16)
    for s in range(4):
        w1[32 * s:32 * s + 19, :] = W1.astype(ml_dtypes.bfloat16)
    w2 = np.empty((128, H), ml_dtypes.bfloat16)
    w2[0:64] = W2.astype(ml_dtypes.bfloat16)
    w2[64:128] = W2.astype(ml_dtypes.bfloat16)
    w3 = np.zeros((128, 32), ml_dtypes.bfloat16)
    w3[0:64, 0:3] = W3.astype(ml_dtypes.bfloat16)
    w3[64:128, 3:6] = W3.astype(ml_dtypes.bfloat16)
    return w1, w2, w3


def kernel(pos, normal, emb, W1, b1, W2, b2, W3, b3):
    from concourse.bass_utils import run_bass_kernel_spmd

    assert not np.any(b1) and not np.any(b2) and not np.any(b3), (
        "nonzero biases not supported by this kernel build")

    nc = _get_program()
    xt = _pack_inputs(np.asarray(pos), np.asarray(normal), np.asarray(emb),
                      np.asarray(W1))
    w1, w2, w3 = _bake_weights(np.asarray(W1), np.asarray(W2), np.asarray(W3))
    in_maps = [
        {"xt": xt[k], "w1": w1, "w2": w2, "w3": w3}
        for k in range(NC)
    ]
    res = run_bass_kernel_spmd(nc, in_maps, core_ids=list(range(NC)))
    return _unpack(res)


def _unpack(res):
    od = np.stack([res.results[k]["out"] for k in range(NC)])
    # od: [core, g, r, 3e+o, h*L+j]; pair pp = 8g+4h+r; row=(2pp+e)*L+j
    od = od.reshape(NC, GROUPS, 4, 2, 3, 2, L)    # [core, g, r, e, o, h, j]
    od = np.transpose(od, (0, 1, 5, 2, 3, 6, 4))  # [core, g, h, r, e, j, o]
    return np.ascontiguousarray(od.reshape(N, 3))



# revision 2
# speedup vs baseline: 1.4667x; 1.4667x over previous
"""Trainium2 Bass kernel for the NeuralRadiance embedding-lookup MLP.

Contract: kernel(**inputs) takes the FULL inputs from setup_inputs() and
returns the FULL [N, 3] float32 output.

Strategy (data-parallel over 8 NeuronCores):
  host: spatial-hash index computation + table lookup, pack rows into
        transposed bf16 tiles; bake block-diagonal weight tiles.
  device (per core, 262144 rows = 256 pairs of 512-row chunks):
    MM1: one blockdiag matmul per pair (K=51 over two 32-strips, M=128)
         with alternating row-base 0/64 so LDWEIGHTS pulls ahead.
    relu1 on DVE at [128,1024] (two pairs per op).
    MM2: two concurrent M=64 matmuls per pair at (0,0)/(64,64).
    relu2 on ACT (a few slots diverted to DVE to balance engines).
    MM3: K=128 blockdiag [W3;W3] -> [32,512] strips, 8 per group.
    sigmoid on ACT at [128,1024] per 8 pairs.
  PSUM: two pools of 2x[128,1024]; the MM3 accumulator borrows slots
        from them alternately.
"""

import numpy as np
import ml_dtypes

N = 2_097_152
NC = 8
R = N // NC            # rows per core
L = 512                # rows per chunk (matmul free dim)
PAIRS = R // (2 * L)   # 256 pairs per core
SLOTS = PAIRS // 2     # 128 slots (2 pairs each)
MACROS = 32            # input macro-tiles per core ([128, 2048] bf16, 8 pairs)
GROUPS = 32            # sigmoid groups per core (8 pairs each)
TABLE = 32768
FEAT = 16
H = 64

# every REBAL-th slot, relu2 runs on DVE instead of ACT to balance load
REBAL = 13

_cache = {}


def _hash_idx(pos):
    s = (pos * 8.0).astype(np.int32)
    h = (s[:, 0] * np.int32(73856093)) ^ (s[:, 1] * np.int32(19349663)) ^ (
        s[:, 2] * np.int32(83492791))
    return h & np.int32(TABLE - 1)


def _build_program():
    import concourse.bass as bass
    import concourse.bacc as bacc
    import concourse.tile as tile
    from concourse import mybir

    f32 = mybir.dt.float32
    bf16 = mybir.dt.bfloat16
    Act = mybir.ActivationFunctionType

    nc = bacc.Bacc(None, target_bir_lowering=False)
    xt_d = nc.dram_tensor("xt", [MACROS, 128, 2048], bf16, kind="ExternalInput")
    w1_d = nc.dram_tensor("w1", [128, 128], bf16, kind="ExternalInput")
    w2_d = nc.dram_tensor("w2", [128, H], bf16, kind="ExternalInput")
    w3_d = nc.dram_tensor("w3", [128, 32], bf16, kind="ExternalInput")
    out_d = nc.dram_tensor("out", [GROUPS, 4, 6, 2 * L], f32, kind="ExternalOutput")

    with tile.TileContext(nc) as tc:
        with (
            tc.tile_pool(name="wpool", bufs=1) as wpool,
            tc.tile_pool(name="xin", bufs=3) as xin_pool,
            tc.tile_pool(name="h1", bufs=3) as h1_pool,
            tc.tile_pool(name="h2", bufs=6) as h2_pool,
            tc.tile_pool(name="ot", bufs=2) as ot_pool,
            tc.tile_pool(name="psA", bufs=2, space="PSUM") as psA_pool,
            tc.tile_pool(name="psB", bufs=2, space="PSUM") as psB_pool,
        ):
            w1t = wpool.tile([128, 128], bf16)
            nc.sync.dma_start(out=w1t[:], in_=w1_d[:])
            w2t = wpool.tile([128, H], bf16)
            nc.sync.dma_start(out=w2t[:], in_=w2_d[:])
            w3t = wpool.tile([128, 32], bf16)
            nc.sync.dma_start(out=w3t[:], in_=w3_d[:])

            xin_t = {}
            ps1_t, h1_t, ps2_t, h2_t, psO_t = {}, {}, {}, {}, {}

            for t in range(SLOTS + 6):
                # ---- stage 1: input DMA + MM1 (blockdiag, one per pair)
                if t < SLOTS:
                    m = t // 4
                    if t % 4 == 0:
                        xin = xin_pool.tile([128, 2048], bf16, name=f"xin{m}",
                                            tag="xin")
                        nc.sync.dma_start(out=xin[:], in_=xt_d[m])
                        xin_t[m] = xin
                    xin = xin_t[m]
                    ps1 = psA_pool.tile([128, 2 * L], f32, name=f"ps1_{t}",
                                        tag="psA")
                    ps1_t[t] = ps1
                    for u in range(2):
                        p = 2 * t + u          # pair index
                        q = p % 8              # pair within macro
                        B = 64 * (q % 2)       # partition base (rotates LDW)
                        c = q // 2             # column slot in macro tile
                        nc.tensor.matmul(
                            out=ps1[:, u * L:(u + 1) * L],
                            lhsT=w1t[B:B + 51, :],
                            rhs=xin[B:B + 51, c * L:(c + 1) * L],
                            start=True, stop=True,
                            tile_position=(B, 0),
                        )
                # ---- stage 2: relu1 on DVE, [128, 1024]
                if 0 <= t - 1 < SLOTS:
                    s = t - 1
                    h1t = h1_pool.tile([128, 2 * L], bf16, name=f"h1_{s}",
                                       tag="h1")
                    h1_t[s] = h1t
                    nc.vector.tensor_scalar_max(h1t[:], ps1_t.pop(s)[:], 0.0)
                # ---- stage 3: MM2 (two concurrent M=64 matmuls per pair)
                if 0 <= t - 2 < SLOTS:
                    s = t - 2
                    h1t = h1_t[s]
                    ps2 = psB_pool.tile([128, 2 * L], f32, name=f"ps2_{s}",
                                        tag="psB")
                    ps2_t[s] = ps2
                    for u in range(2):
                        sl = slice(u * L, (u + 1) * L)
                        nc.tensor.matmul(
                            out=ps2[0:64, sl],
                            lhsT=w2t[0:64, :],
                            rhs=h1t[0:64, sl],
                            start=True, stop=True,
                            tile_position=(0, 0),
                        )
                        nc.tensor.matmul(
                            out=ps2[64:128, sl],
                            lhsT=w2t[64:128, :],
                            rhs=h1t[64:128, sl],
                            start=True, stop=True,
                            tile_position=(64, 64),
                        )
                    del h1_t[s]
                # ---- stage 4: relu2 on ACT (sometimes DVE for balance)
                if 0 <= t - 3 < SLOTS:
                    s = t - 3
                    h2t = h2_pool.tile([128, 2 * L], bf16, name=f"h2_{s}",
                                       tag="h2")
                    h2_t[s] = h2t
                    if s % REBAL == REBAL - 1:
                        nc.vector.tensor_scalar_max(h2t[:], ps2_t.pop(s)[:],
                                                    0.0)
                    else:
                        nc.scalar.activation(h2t[:], ps2_t.pop(s)[:], Act.Relu)
                # ---- stage 5: MM3 wave (8 pairs -> one [128,1024] psO)
                if t - 7 >= 0 and (t - 7) % 4 == 0 and (t - 7) // 4 < GROUPS:
                    j = (t - 7) // 4
                    pool = psA_pool if j % 2 == 0 else psB_pool
                    tag = "psA" if j % 2 == 0 else "psB"
                    psO = pool.tile([128, 2 * L], f32, name=f"psO_{j}", tag=tag)
                    psO_t[j] = psO
                    for qq in range(8):
                        p = 8 * j + qq
                        r, hh = qq % 4, qq // 4
                        s = p // 2
                        half = p % 2
                        nc.tensor.matmul(
                            out=psO[32 * r:32 * r + 32,
                                    hh * L:(hh + 1) * L],
                            lhsT=w3t[:],
                            rhs=h2_t[s][:, half * L:(half + 1) * L],
                            start=True, stop=True,
                            tile_position=(0, 32 * r),
                        )
                        if half == 1:
                            del h2_t[s]
                # ---- stage 6: sigmoid + output DMAs
                if t - 8 >= 0 and (t - 8) % 4 == 0 and (t - 8) // 4 < GROUPS:
                    j = (t - 8) // 4
                    otile = ot_pool.tile([128, 2 * L], f32, name=f"ot_{j}",
                                         tag="ot")
                    nc.scalar.activation(otile[:], psO_t.pop(j)[:], Act.Sigmoid)
                    for rr in range(4):
                        nc.sync.dma_start(
                            out=out_d[j, rr],
                            in_=otile[32 * rr:32 * rr + 6, :],
                        )
    nc.finalize()
    return nc


def _get_program():
    if "nc" not in _cache:
        _cache["nc"] = _build_program()
    return _cache["nc"]


def _pack_inputs(pos, normal, emb, W1):
    """Host-side: hash + table lookup + bake transposed bf16 tiles.

    pair p (q = p%8 in macro): partition base 64*(q%2) + 32*e, col slot
    q//2; chunks 2p (e=0) and 2p+1 (e=1)."""
    idx = _hash_idx(pos)
    x19 = np.empty((N, 19), np.float32)
    x19[:, :FEAT] = emb[idx]
    x19[:, FEAT:] = normal
    xv = x19.astype(ml_dtypes.bfloat16)
    r = xv.reshape(NC, MACROS, 8, 2, L, 19)     # [k, m, q, e, j, f]
    xt = np.zeros((NC, MACROS, 2, 2, 32, 4, L), ml_dtypes.bfloat16)
    for q in range(8):
        a, c = q % 2, q // 2
        for e in range(2):
            xt[:, :, a, e, :19, c, :] = r[:, :, q, e].transpose(0, 1, 3, 2)
    return xt.reshape(NC, MACROS, 128, 2048)


def _bake_weights(W1, W2, W3):
    w1 = np.zeros((128, 128), ml_dtypes.bfloat16)
    for base in (0, 64):
        w1[base + 0:base + 19, 0:64] = W1.astype(ml_dtypes.bfloat16)
        w1[base + 32:base + 51, 64:128] = W1.astype(ml_dtypes.bfloat16)
    w2 = np.empty((128, H), ml_dtypes.bfloat16)
    w2[0:64] = W2.astype(ml_dtypes.bfloat16)
    w2[64:128] = W2.astype(ml_dtypes.bfloat16)
    w3 = np.zeros((128, 32), ml_dtypes.bfloat16)
    w3[0:64, 0:3] = W3.astype(ml_dtypes.bfloat16)
    w3[64:128, 3:6] = W3.astype(ml_dtypes.bfloat16)
    return w1, w2, w3


def kernel(pos, normal, emb, W1, b1, W2, b2, W3, b3):
    from concourse.bass_utils import run_bass_kernel_spmd

    assert not np.any(b1) and not np.any(b2) and not np.any(b3), (
        "nonzero biases not supported by this kernel build")

    nc = _get_program()
    xt = _pack_inputs(np.asarray(pos), np.asarray(normal), np.asarray(emb),
                      np.asarray(W1))
    w1, w2, w3 = _bake_weights(np.asarray(W1), np.asarray(W2), np.asarray(W3))
    in_maps = [
        {"xt": xt[k], "w1": w1, "w2": w2, "w3": w3}
        for k in range(NC)
    ]
    res = run_bass_kernel_spmd(nc, in_maps, core_ids=list(range(NC)))
    return _unpack(res)


def _unpack(res):
    od = np.stack([res.results[k]["out"] for k in range(NC)])
    # od: [core, g, r, 3e+o, h*L+j]; pair p = 8g+4h+r; row=(2p+e)*L+j
    od = od.reshape(NC, GROUPS, 4, 2, 3, 2, L)    # [core, g, r, e, o, h, j]
    od = np.transpose(od, (0, 1, 5, 2, 3, 6, 4))  # [core, g, h, r, e, j, o]
    return np.ascontiguousarray(od.reshape(N, 3))


# revision 19
# speedup vs baseline: 1.6994x; 1.1587x over previous
"""Trainium2 Bass kernel for the NeuralRadiance embedding-lookup MLP.

Contract: kernel(**inputs) takes the FULL inputs from setup_inputs() and
returns the FULL [N, 3] float32 output.

Strategy (data-parallel over 8 NeuronCores):
  host: spatial-hash index computation + table lookup, pack rows into
        transposed bf16 tiles; bake block-diagonal weight tiles.
  device (per core, 262144 rows = 256 pairs of 512-row chunks):
    MM1: one blockdiag matmul per pair (K=51 over two 32-strips, M=128)
         with alternating row-base 0/64 so LDWEIGHTS pulls ahead.
    relu1 on DVE at [128,1024] (two pairs per op).
    MM2: two concurrent M=64 matmuls per pair at (0,0)/(64,64).
    relu2 on ACT (a few slots diverted to DVE to balance engines).
    MM3: K=128 blockdiag [W3;W3] -> [32,512] strips, 8 per group.
    sigmoid on ACT at [128,1024] per 8 pairs.
  PSUM: two pools of 2x[128,1024]; the MM3 accumulator borrows slots
        from them alternately.
"""

import numpy as np
import ml_dtypes

N = 2_097_152
NC = 8
R = N // NC            # rows per core
L = 512                # rows per chunk (matmul free dim)
PAIRS = R // (2 * L)   # 256 pairs per core
SLOTS = PAIRS // 2     # 128 slots (2 pairs each)
MACROS = 32            # input macro-tiles per core ([128, 2048] bf16, 8 pairs)
GROUPS = 32            # sigmoid groups per core (8 pairs each)
TABLE = 32768
FEAT = 16
H = 64

# every REBAL-th slot, relu2 runs on DVE instead of ACT to balance load
REBAL = 13

_cache = {}


def _hash_idx(pos):
    s = (pos * 8.0).astype(np.int32)
    h = (s[:, 0] * np.int32(73856093)) ^ (s[:, 1] * np.int32(19349663)) ^ (
        s[:, 2] * np.int32(83492791))
    return h & np.int32(TABLE - 1)


def _build_program():
    import concourse.bass as bass
    import concourse.bacc as bacc
    import concourse.tile as tile
    from concourse import mybir

    f32 = mybir.dt.float32
    bf16 = mybir.dt.bfloat16
    Act = mybir.ActivationFunctionType

    nc = bacc.Bacc(None, target_bir_lowering=False)
    # sparse macro tiles (full 128-partition DMAs are descriptor-cheap):
    # partition 64a + 19e + f, zeros at 38-63/102-127
    xt_d = nc.dram_tensor("xt", [MACROS, 128, 2048], bf16,
                          kind="ExternalInput")
    w1_d = nc.dram_tensor("w1", [128, 128], bf16, kind="ExternalInput")
    w2_d = nc.dram_tensor("w2", [128, H], bf16, kind="ExternalInput")
    w3_d = nc.dram_tensor("w3", [128, 32], bf16, kind="ExternalInput")
    # batched bf16 output: one [128, 4096] buffer per 4 sigmoid groups
    out_d = nc.dram_tensor("out", [GROUPS // 4, 128, 4096], bf16,
                           kind="ExternalOutput")

    with tile.TileContext(nc) as tc:
        with (
            tc.tile_pool(name="wpool", bufs=1) as wpool,
            tc.tile_pool(name="xin", bufs=4) as xin_pool,
            tc.tile_pool(name="h1", bufs=3) as h1_pool,
            tc.tile_pool(name="h2", bufs=6) as h2_pool,
            tc.tile_pool(name="ot", bufs=2) as ot_pool,
            tc.tile_pool(name="psA", bufs=2, space="PSUM") as psA_pool,
            tc.tile_pool(name="psB", bufs=2, space="PSUM") as psB_pool,
        ):
            w1t = wpool.tile([128, 128], bf16)
            nc.sync.dma_start(out=w1t[:], in_=w1_d[:])
            w2t = wpool.tile([128, H], bf16)
            nc.sync.dma_start(out=w2t[:], in_=w2_d[:])
            w3t = wpool.tile([128, 32], bf16)
            nc.sync.dma_start(out=w3t[:], in_=w3_d[:])

            # dummy activations: pull both ACT table loads to kernel start
            warm = wpool.tile([128, 8], f32)
            nc.scalar.activation(warm[:], warm[:], Act.Relu)
            nc.scalar.activation(warm[:], warm[:], Act.Sigmoid)

            xin_t = {}
            ps1_t, h1_t, ps2_t, h2_t, psO_t, obuf_t = {}, {}, {}, {}, {}, {}

            for t in range(SLOTS + 6):
                # ---- stage 1: input DMA + MM1 (blockdiag, one per pair)
                if t < SLOTS:
                    m = t // 4
                    if t % 4 == 0:
                        xin = xin_pool.tile([128, 2048], bf16, name=f"xin{m}",
                                            tag="xin")
                        nc.sync.dma_start(out=xin[:], in_=xt_d[m])
                        xin_t[m] = xin
                    xin = xin_t[m]
                    ps1 = psA_pool.tile([128, 2 * L], f32, name=f"ps1_{t}",
                                        tag="psA")
                    ps1_t[t] = ps1
                    for u in range(2):
                        p = 2 * t + u          # pair index
                        q = p % 8              # pair within macro
                        B = 64 * (q % 2)       # partition base (rotates LDW)
                        c = q // 2             # column slot in macro tile
                        nc.tensor.matmul(
                            out=ps1[:, u * L:(u + 1) * L],
                            lhsT=w1t[B:B + 38, :],
                            rhs=xin[B:B + 38, c * L:(c + 1) * L],
                            start=True, stop=True,
                            tile_position=(B, 0),
                        )
                # ---- stage 2: relu1 on DVE, [128, 1024]
                if 0 <= t - 1 < SLOTS:
                    s = t - 1
                    h1t = h1_pool.tile([128, 2 * L], bf16, name=f"h1_{s}",
                                       tag="h1")
                    h1_t[s] = h1t
                    nc.vector.tensor_scalar_max(h1t[:], ps1_t.pop(s)[:], 0.0)
                # ---- stage 3: MM2 (two concurrent M=64 matmuls per pair)
                if 0 <= t - 2 < SLOTS:
                    s = t - 2
                    h1t = h1_t[s]
                    ps2 = psB_pool.tile([128, 2 * L], f32, name=f"ps2_{s}",
                                        tag="psB")
                    ps2_t[s] = ps2
                    for u in range(2):
                        sl = slice(u * L, (u + 1) * L)
                        nc.tensor.matmul(
                            out=ps2[0:64, sl],
                            lhsT=w2t[0:64, :],
                            rhs=h1t[0:64, sl],
                            start=True, stop=True,
                            tile_position=(0, 0),
                        )
                        nc.tensor.matmul(
                            out=ps2[64:128, sl],
                            lhsT=w2t[64:128, :],
                            rhs=h1t[64:128, sl],
                            start=True, stop=True,
                            tile_position=(64, 64),
                        )
                    del h1_t[s]
                # ---- stage 4: relu2 on ACT (sometimes DVE for balance)
                if 0 <= t - 3 < SLOTS:
                    s = t - 3
                    h2t = h2_pool.tile([128, 2 * L], bf16, name=f"h2_{s}",
                                       tag="h2")
                    h2_t[s] = h2t
                    if s % REBAL == REBAL - 1:
                        nc.vector.tensor_scalar_max(h2t[:], ps2_t.pop(s)[:],
                                                    0.0)
                    else:
                        nc.scalar.activation(h2t[:], ps2_t.pop(s)[:], Act.Relu)
                # ---- stage 5: MM3 wave (8 pairs -> one [128,1024] psO)
                if t - 7 >= 0 and (t - 7) % 4 == 0 and (t - 7) // 4 < GROUPS:
                    j = (t - 7) // 4
                    pool = psA_pool if j % 2 == 0 else psB_pool
                    tag = "psA" if j % 2 == 0 else "psB"
                    psO = pool.tile([128, 2 * L], f32, name=f"psO_{j}", tag=tag)
                    psO_t[j] = psO
                    for qq in range(8):
                        p = 8 * j + qq
                        r, hh = qq % 4, qq // 4
                        s = p // 2
                        half = p % 2
                        nc.tensor.matmul(
                            out=psO[32 * r:32 * r + 32,
                                    hh * L:(hh + 1) * L],
                            lhsT=w3t[:],
                            rhs=h2_t[s][:, half * L:(half + 1) * L],
                            start=True, stop=True,
                            tile_position=(0, 32 * r),
                        )
                        if half == 1:
                            del h2_t[s]
                # ---- stage 6: sigmoid into batched obuf + DMA per 4 groups
                if t - 8 >= 0 and (t - 8) % 4 == 0 and (t - 8) // 4 < GROUPS:
                    j = (t - 8) // 4
                    if j % 4 == 0:
                        obuf = ot_pool.tile([128, 4096], bf16, name=f"ob{j//4}",
                                            tag="ot")
                        obuf_t[j // 4] = obuf
                    obuf = obuf_t[j // 4]
                    jj = j % 4
                    nc.scalar.activation(obuf[:, jj * 1024:(jj + 1) * 1024],
                                         psO_t.pop(j)[:], Act.Sigmoid)
                    if jj == 3:
                        nc.sync.dma_start(out=out_d[j // 4],
                                          in_=obuf_t.pop(j // 4)[:])
    nc.finalize()
    return nc


def _get_program():
    if "nc" not in _cache:
        _cache["nc"] = _build_program()
    return _cache["nc"]


def _pack_inputs(pos, normal, emb, W1):
    """Host-side: hash + table lookup + bake transposed bf16 tiles.

    pair p (q = p%8 in macro): partition base 64*(q%2) + 32*e, col slot
    q//2; chunks 2p (e=0) and 2p+1 (e=1)."""
    idx = _hash_idx(pos)
    x19 = np.empty((N, 19), np.float32)
    x19[:, :FEAT] = emb[idx]
    x19[:, FEAT:] = normal
    xv = x19.astype(ml_dtypes.bfloat16)
    r = xv.reshape(NC, MACROS, 8, 2, L, 19)     # [k, m, q, e, j, f]
    xt = np.zeros((NC, MACROS, 2, 64, 4, L), ml_dtypes.bfloat16)
    for q in range(8):
        a, c = q % 2, q // 2
        for e in range(2):
            xt[:, :, a, 19 * e:19 * e + 19, c, :] = (
                r[:, :, q, e].transpose(0, 1, 3, 2))
    return xt.reshape(NC, MACROS, 128, 2048)


def _bake_weights(W1, W2, W3):
    w1 = np.zeros((128, 128), ml_dtypes.bfloat16)
    for base in (0, 64):
        w1[base + 0:base + 19, 0:64] = W1.astype(ml_dtypes.bfloat16)
        w1[base + 19:base + 38, 64:128] = W1.astype(ml_dtypes.bfloat16)
    w2 = np.empty((128, H), ml_dtypes.bfloat16)
    w2[0:64] = W2.astype(ml_dtypes.bfloat16)
    w2[64:128] = W2.astype(ml_dtypes.bfloat16)
    w3 = np.zeros((128, 32), ml_dtypes.bfloat16)
    w3[0:64, 0:3] = W3.astype(ml_dtypes.bfloat16)
    w3[64:128, 3:6] = W3.astype(ml_dtypes.bfloat16)
    return w1, w2, w3


def kernel(pos, normal, emb, W1, b1, W2, b2, W3, b3):
    from concourse.bass_utils import run_bass_kernel_spmd

    assert not np.any(b1) and not np.any(b2) and not np.any(b3), (
        "nonzero biases not supported by this kernel build")

    nc = _get_program()
    xt = _pack_inputs(np.asarray(pos), np.asarray(normal), np.asarray(emb),
                      np.asarray(W1))
    w1, w2, w3 = _bake_weights(np.asarray(W1), np.asarray(W2), np.asarray(W3))
    in_maps = [
        {"xt": xt[k], "w1": w1, "w2": w2, "w3": w3}
        for k in range(NC)
    ]
    res = run_bass_kernel_spmd(nc, in_maps, core_ids=list(range(NC)))
    return _unpack(res)


def _unpack(res):
    od = np.stack([np.asarray(res.results[k]["out"]) for k in range(NC)])
    # od: [core, G, 32r+o, (jj, h, j2)]; pair p = 8*(4G+jj)+4h+r
    od = od.reshape(NC, GROUPS // 4, 4, 32, 4, 2, L)[:, :, :, 0:6]
    od = od.reshape(NC, GROUPS // 4, 4, 2, 3, 4, 2, L)
    # dims: [k, G, r, e, c, jj, h, j2] -> [k, G, jj, h, r, e, j2, c]
    od = np.transpose(od, (0, 1, 5, 6, 2, 3, 7, 4))
    return np.ascontiguousarray(od.reshape(N, 3).astype(np.float32))


# revision 24
# speedup vs baseline: 1.7205x; 1.0124x over previous
"""Trainium2 Bass kernel for the NeuralRadiance embedding-lookup MLP.

Contract: kernel(**inputs) takes the FULL inputs from setup_inputs() and
returns the FULL [N, 3] float32 output.

Strategy (data-parallel over 8 NeuronCores):
  host: spatial-hash index computation + table lookup, pack rows into
        transposed bf16 tiles; bake block-diagonal weight tiles.
  device (per core, 262144 rows = 256 pairs of 512-row chunks):
    MM1: one blockdiag matmul per pair (K=51 over two 32-strips, M=128)
         with alternating row-base 0/64 so LDWEIGHTS pulls ahead.
    relu1 on DVE at [128,1024] (two pairs per op).
    MM2: two concurrent M=64 matmuls per pair at (0,0)/(64,64).
    relu2 on ACT (a few slots diverted to DVE to balance engines).
    MM3: K=128 blockdiag [W3;W3] -> [32,512] strips, 8 per group.
    sigmoid on ACT at [128,1024] per 8 pairs.
  PSUM: two pools of 2x[128,1024]; the MM3 accumulator borrows slots
        from them alternately.
"""

import numpy as np
import ml_dtypes

N = 2_097_152
NC = 8
R = N // NC            # rows per core
L = 512                # rows per chunk (matmul free dim)
PAIRS = R // (2 * L)   # 256 pairs per core
SLOTS = PAIRS // 2     # 128 slots (2 pairs each)
MACROS = 32            # input macro-tiles per core ([128, 2048] bf16, 8 pairs)
GROUPS = 32            # sigmoid groups per core (8 pairs each)
TABLE = 32768
FEAT = 16
H = 64

# every REBAL-th slot, relu2 runs on DVE instead of ACT to balance load
REBAL = 11

_cache = {}


def _hash_idx(pos):
    s = (pos * 8.0).astype(np.int32)
    h = (s[:, 0] * np.int32(73856093)) ^ (s[:, 1] * np.int32(19349663)) ^ (
        s[:, 2] * np.int32(83492791))
    return h & np.int32(TABLE - 1)


def _build_program():
    import concourse.bass as bass
    import concourse.bacc as bacc
    import concourse.tile as tile
    from concourse import mybir

    f32 = mybir.dt.float32
    bf16 = mybir.dt.bfloat16
    Act = mybir.ActivationFunctionType

    nc = bacc.Bacc(None, target_bir_lowering=False)
    # sparse macro tiles (full 128-partition DMAs are descriptor-cheap):
    # partition 64a + 19e + f, zeros at 38-63/102-127
    xt_d = nc.dram_tensor("xt", [MACROS, 128, 2048], bf16,
                          kind="ExternalInput")
    w1_d = nc.dram_tensor("w1", [128, 128], bf16, kind="ExternalInput")
    w2_d = nc.dram_tensor("w2", [128, H], bf16, kind="ExternalInput")
    w3_d = nc.dram_tensor("w3", [128, 32], bf16, kind="ExternalInput")
    # bf16 output: one [128, 1024] tile per sigmoid group
    out_d = nc.dram_tensor("out", [GROUPS, 128, 1024], bf16,
                           kind="ExternalOutput")

    with tile.TileContext(nc) as tc:
        with (
            tc.tile_pool(name="wpool", bufs=1) as wpool,
            tc.tile_pool(name="xin", bufs=4) as xin_pool,
            tc.tile_pool(name="h1", bufs=3) as h1_pool,
            tc.tile_pool(name="h2", bufs=6) as h2_pool,
            tc.tile_pool(name="ot", bufs=3) as ot_pool,
            tc.tile_pool(name="psA", bufs=2, space="PSUM") as psA_pool,
            tc.tile_pool(name="psB", bufs=2, space="PSUM") as psB_pool,
        ):
            w1t = wpool.tile([128, 128], bf16)
            nc.sync.dma_start(out=w1t[:], in_=w1_d[:])
            w2t = wpool.tile([128, H], bf16)
            nc.sync.dma_start(out=w2t[:], in_=w2_d[:])
            w3t = wpool.tile([128, 32], bf16)
            nc.sync.dma_start(out=w3t[:], in_=w3_d[:])

            # dummy activations: pull both ACT table loads to kernel start
            warm = wpool.tile([128, 8], f32)
            nc.scalar.activation(warm[:], warm[:], Act.Relu)
            nc.scalar.activation(warm[:], warm[:], Act.Sigmoid)

            xin_t = {}
            ps1_t, h1_t, ps2_t, h2_t, psO_t, obuf_t = {}, {}, {}, {}, {}, {}

            for t in range(SLOTS + 6):
                # ---- stage 1: input DMA + MM1 (blockdiag, one per pair)
                if t < SLOTS:
                    m = t // 4
                    if t % 4 == 0:
                        xin = xin_pool.tile([128, 2048], bf16, name=f"xin{m}",
                                            tag="xin")
                        nc.sync.dma_start(out=xin[:], in_=xt_d[m])
                        xin_t[m] = xin
                    xin = xin_t[m]
                    ps1 = psA_pool.tile([128, 2 * L], f32, name=f"ps1_{t}",
                                        tag="psA")
                    ps1_t[t] = ps1
                    for u in range(2):
                        p = 2 * t + u          # pair index
                        q = p % 8              # pair within macro
                        B = 64 * (q % 2)       # partition base (rotates LDW)
                        c = q // 2             # column slot in macro tile
                        nc.tensor.matmul(
                            out=ps1[:, u * L:(u + 1) * L],
                            lhsT=w1t[B:B + 38, :],
                            rhs=xin[B:B + 38, c * L:(c + 1) * L],
                            start=True, stop=True,
                            tile_position=(B, 0),
                        )
                # ---- stage 2: relu1 on DVE, [128, 1024]
                if 0 <= t - 1 < SLOTS:
                    s = t - 1
                    h1t = h1_pool.tile([128, 2 * L], bf16, name=f"h1_{s}",
                                       tag="h1")
                    h1_t[s] = h1t
                    nc.vector.tensor_scalar_max(h1t[:], ps1_t.pop(s)[:], 0.0)
                # ---- stage 3: MM2 (two concurrent M=64 matmuls per pair)
                if 0 <= t - 2 < SLOTS:
                    s = t - 2
                    h1t = h1_t[s]
                    ps2 = psB_pool.tile([128, 2 * L], f32, name=f"ps2_{s}",
                                        tag="psB")
                    ps2_t[s] = ps2
                    for u in range(2):
                        sl = slice(u * L, (u + 1) * L)
                        nc.tensor.matmul(
                            out=ps2[0:64, sl],
                            lhsT=w2t[0:64, :],
                            rhs=h1t[0:64, sl],
                            start=True, stop=True,
                            tile_position=(0, 0),
                        )
                        nc.tensor.matmul(
                            out=ps2[64:128, sl],
                            lhsT=w2t[64:128, :],
                            rhs=h1t[64:128, sl],
                            start=True, stop=True,
                            tile_position=(64, 64),
                        )
                    del h1_t[s]
                # ---- stage 4: relu2 on ACT (sometimes DVE for balance)
                if 0 <= t - 3 < SLOTS:
                    s = t - 3
                    h2t = h2_pool.tile([128, 2 * L], bf16, name=f"h2_{s}",
                                       tag="h2")
                    h2_t[s] = h2t
                    if s % REBAL == REBAL - 1:
                        nc.vector.tensor_scalar_max(h2t[:], ps2_t.pop(s)[:],
                                                    0.0)
                    else:
                        nc.scalar.activation(h2t[:], ps2_t.pop(s)[:], Act.Relu)
                # ---- stage 5: MM3 wave (8 pairs -> one [128,1024] psO)
                if t - 7 >= 0 and (t - 7) % 4 == 0 and (t - 7) // 4 < GROUPS:
                    j = (t - 7) // 4
                    pool = psA_pool if j % 2 == 0 else psB_pool
                    tag = "psA" if j % 2 == 0 else "psB"
                    psO = pool.tile([128, 2 * L], f32, name=f"psO_{j}", tag=tag)
                    psO_t[j] = psO
                    for qq in range(8):
                        p = 8 * j + qq
                        r, hh = qq % 4, qq // 4
                        s = p // 2
                        half = p % 2
                        nc.tensor.matmul(
                            out=psO[32 * r:32 * r + 32,
                                    hh * L:(hh + 1) * L],
                            lhsT=w3t[:],
                            rhs=h2_t[s][:, half * L:(half + 1) * L],
                            start=True, stop=True,
                            tile_position=(0, 32 * r),
                        )
                        if half == 1:
                            del h2_t[s]
                # ---- stage 6: sigmoid + one output DMA per group
                if t - 8 >= 0 and (t - 8) % 4 == 0 and (t - 8) // 4 < GROUPS:
                    j = (t - 8) // 4
                    obuf = ot_pool.tile([128, 2 * L], bf16, name=f"ob{j}",
                                        tag="ot")
                    nc.scalar.activation(obuf[:], psO_t.pop(j)[:], Act.Sigmoid)
                    nc.sync.dma_start(out=out_d[j], in_=obuf[:])
    nc.finalize()
    return nc


def _get_program():
    if "nc" not in _cache:
        _cache["nc"] = _build_program()
    return _cache["nc"]


def _pack_inputs(pos, normal, emb, W1):
    """Host-side: hash + table lookup + bake transposed bf16 tiles.

    pair p (q = p%8 in macro): partition base 64*(q%2) + 32*e, col slot
    q//2; chunks 2p (e=0) and 2p+1 (e=1)."""
    idx = _hash_idx(pos)
    x19 = np.empty((N, 19), np.float32)
    x19[:, :FEAT] = emb[idx]
    x19[:, FEAT:] = normal
    xv = x19.astype(ml_dtypes.bfloat16)
    r = xv.reshape(NC, MACROS, 8, 2, L, 19)     # [k, m, q, e, j, f]
    xt = np.zeros((NC, MACROS, 2, 64, 4, L), ml_dtypes.bfloat16)
    for q in range(8):
        a, c = q % 2, q // 2
        for e in range(2):
            xt[:, :, a, 19 * e:19 * e + 19, c, :] = (
                r[:, :, q, e].transpose(0, 1, 3, 2))
    return xt.reshape(NC, MACROS, 128, 2048)


def _bake_weights(W1, W2, W3):
    w1 = np.zeros((128, 128), ml_dtypes.bfloat16)
    for base in (0, 64):
        w1[base + 0:base + 19, 0:64] = W1.astype(ml_dtypes.bfloat16)
        w1[base + 19:base + 38, 64:128] = W1.astype(ml_dtypes.bfloat16)
    w2 = np.empty((128, H), ml_dtypes.bfloat16)
    w2[0:64] = W2.astype(ml_dtypes.bfloat16)
    w2[64:128] = W2.astype(ml_dtypes.bfloat16)
    w3 = np.zeros((128, 32), ml_dtypes.bfloat16)
    w3[0:64, 0:3] = W3.astype(ml_dtypes.bfloat16)
    w3[64:128, 3:6] = W3.astype(ml_dtypes.bfloat16)
    return w1, w2, w3


def kernel(pos, normal, emb, W1, b1, W2, b2, W3, b3):
    from concourse.bass_utils import run_bass_kernel_spmd

    assert not np.any(b1) and not np.any(b2) and not np.any(b3), (
        "nonzero biases not supported by this kernel build")

    nc = _get_program()
    xt = _pack_inputs(np.asarray(pos), np.asarray(normal), np.asarray(emb),
                      np.asarray(W1))
    w1, w2, w3 = _bake_weights(np.asarray(W1), np.asarray(W2), np.asarray(W3))
    in_maps = [
        {"xt": xt[k], "w1": w1, "w2": w2, "w3": w3}
        for k in range(NC)
    ]
    res = run_bass_kernel_spmd(nc, in_maps, core_ids=list(range(NC)))
    return _unpack(res)


def _unpack(res):
    od = np.stack([np.asarray(res.results[k]["out"]) for k in range(NC)])
    # od: [core, g, 32r+o, h*L+j2]; pair p = 8g+4h+r; row=(2p+e)*L+j2
    od = od.reshape(NC, GROUPS, 4, 32, 2, L)[:, :, :, 0:6]
    od = od.reshape(NC, GROUPS, 4, 2, 3, 2, L)   # [k, g, r, e, c, h, j2]
    od = np.transpose(od, (0, 1, 5, 2, 3, 6, 4))  # [k, g, h, r, e, j2, c]
    return np.ascontiguousarray(od.reshape(N, 3).astype(np.float32))


# revision 26
# speedup vs baseline: 1.7396x; 1.0111x over previous
"""Trainium2 Bass kernel for the NeuralRadiance embedding-lookup MLP.

Contract: kernel(**inputs) takes the FULL inputs from setup_inputs() and
returns the FULL [N, 3] float32 output.

Strategy (data-parallel over 8 NeuronCores):
  host: spatial-hash index computation + table lookup, pack rows into
        transposed bf16 tiles; bake block-diagonal weight tiles.
  device (per core, 262144 rows = 256 pairs of 512-row chunks):
    MM1: one blockdiag matmul per pair (K=51 over two 32-strips, M=128)
         with alternating row-base 0/64 so LDWEIGHTS pulls ahead.
    relu1 on DVE at [128,1024] (two pairs per op).
    MM2: two concurrent M=64 matmuls per pair at (0,0)/(64,64).
    relu2 on ACT (a few slots diverted to DVE to balance engines).
    MM3: K=128 blockdiag [W3;W3] -> [32,512] strips, 8 per group.
    sigmoid on ACT at [128,1024] per 8 pairs.
  PSUM: two pools of 2x[128,1024]; the MM3 accumulator borrows slots
        from them alternately.
"""

import numpy as np
import ml_dtypes

N = 2_097_152
NC = 8
R = N // NC            # rows per core
L = 512                # rows per chunk (matmul free dim)
PAIRS = R // (2 * L)   # 256 pairs per core
SLOTS = PAIRS // 2     # 128 slots (2 pairs each)
MACROS = 32            # input macro-tiles per core ([128, 2048] bf16, 8 pairs)
GROUPS = 32            # sigmoid groups per core (8 pairs each)
TABLE = 32768
FEAT = 16
H = 64

# every REBAL-th slot, relu2 runs on DVE instead of ACT to balance load
REBAL = 11

_cache = {}


def _hash_idx(pos):
    s = (pos * 8.0).astype(np.int32)
    h = (s[:, 0] * np.int32(73856093)) ^ (s[:, 1] * np.int32(19349663)) ^ (
        s[:, 2] * np.int32(83492791))
    return h & np.int32(TABLE - 1)


def _build_program():
    import concourse.bass as bass
    import concourse.bacc as bacc
    import concourse.tile as tile
    from concourse import mybir

    f32 = mybir.dt.float32
    bf16 = mybir.dt.bfloat16
    Act = mybir.ActivationFunctionType

    nc = bacc.Bacc(None, target_bir_lowering=False)
    # sparse macro tiles (full 128-partition DMAs are descriptor-cheap):
    # partition 64a + 19e + f, zeros at 38-63/102-127
    xt_d = nc.dram_tensor("xt", [MACROS, 128, 2048], bf16,
                          kind="ExternalInput")
    w1_d = nc.dram_tensor("w1", [128, 128], bf16, kind="ExternalInput")
    w2_d = nc.dram_tensor("w2", [128, H], bf16, kind="ExternalInput")
    w3_d = nc.dram_tensor("w3", [128, 32], bf16, kind="ExternalInput")
    # bf16 output: one [128, 1024] tile per sigmoid group
    out_d = nc.dram_tensor("out", [GROUPS, 128, 1024], bf16,
                           kind="ExternalOutput")

    with tile.TileContext(nc) as tc:
        with (
            tc.tile_pool(name="wpool", bufs=1) as wpool,
            tc.tile_pool(name="xin", bufs=4) as xin_pool,
            tc.tile_pool(name="h1", bufs=3) as h1_pool,
            tc.tile_pool(name="h2", bufs=6) as h2_pool,
            tc.tile_pool(name="ot", bufs=3) as ot_pool,
            tc.tile_pool(name="psA", bufs=2, space="PSUM") as psA_pool,
            tc.tile_pool(name="psB", bufs=2, space="PSUM") as psB_pool,
        ):
            w1t = wpool.tile([128, 128], bf16)
            nc.sync.dma_start(out=w1t[:], in_=w1_d[:])
            w2t = wpool.tile([128, H], bf16)
            nc.sync.dma_start(out=w2t[:], in_=w2_d[:])
            w3t = wpool.tile([128, 32], bf16)
            nc.sync.dma_start(out=w3t[:], in_=w3_d[:])

            # dummy activations: pull both ACT table loads to kernel start
            warm = wpool.tile([128, 8], f32)
            nc.scalar.activation(warm[:], warm[:], Act.Relu)
            nc.scalar.activation(warm[:], warm[:], Act.Sigmoid)

            xin_t = {}
            ps1_t, h1_t, ps2_t, h2_t, psO_t, obuf_t = {}, {}, {}, {}, {}, {}

            for t in range(SLOTS + 6):
                # ---- stage 5: MM3 wave (8 pairs -> one [128,1024] psO)
                # emitted first so the borrowed-slot request outranks MM1's
                if t - 7 >= 0 and (t - 7) % 4 == 0 and (t - 7) // 4 < GROUPS:
                    j = (t - 7) // 4
                    pool = psA_pool if j % 2 == 0 else psB_pool
                    tag = "psA" if j % 2 == 0 else "psB"
                    psO = pool.tile([128, 2 * L], f32, name=f"psO_{j}", tag=tag)
                    psO_t[j] = psO
                    for qq in range(8):
                        p = 8 * j + qq
                        r, hh = qq % 4, qq // 4
                        s = p // 2
                        half = p % 2
                        nc.tensor.matmul(
                            out=psO[32 * r:32 * r + 32,
                                    hh * L:(hh + 1) * L],
                            lhsT=w3t[:],
                            rhs=h2_t[s][:, half * L:(half + 1) * L],
                            start=True, stop=True,
                            tile_position=(0, 32 * r),
                        )
                        if half == 1:
                            del h2_t[s]
                # ---- stage 6: sigmoid + one output DMA per group
                if t - 8 >= 0 and (t - 8) % 4 == 0 and (t - 8) // 4 < GROUPS:
                    j = (t - 8) // 4
                    obuf = ot_pool.tile([128, 2 * L], bf16, name=f"ob{j}",
                                        tag="ot")
                    nc.scalar.activation(obuf[:], psO_t.pop(j)[:], Act.Sigmoid)
                    nc.sync.dma_start(out=out_d[j], in_=obuf[:])
                # ---- stage 1: input DMA + MM1 (blockdiag, one per pair)
                if t < SLOTS:
                    m = t // 4
                    if t % 4 == 0:
                        xin = xin_pool.tile([128, 2048], bf16, name=f"xin{m}",
                                            tag="xin")
                        nc.sync.dma_start(out=xin[:], in_=xt_d[m])
                        xin_t[m] = xin
                    xin = xin_t[m]
                    ps1 = psA_pool.tile([128, 2 * L], f32, name=f"ps1_{t}",
                                        tag="psA")
                    ps1_t[t] = ps1
                    for u in range(2):
                        p = 2 * t + u          # pair index
                        q = p % 8              # pair within macro
                        B = 64 * (q % 2)       # partition base (rotates LDW)
                        c = q // 2             # column slot in macro tile
                        nc.tensor.matmul(
                            out=ps1[:, u * L:(u + 1) * L],
                            lhsT=w1t[B:B + 38, :],
                            rhs=xin[B:B + 38, c * L:(c + 1) * L],
                            start=True, stop=True,
                            tile_position=(B, 0),
                        )
                # ---- stage 2: relu1 on DVE, [128, 1024]
                if 0 <= t - 1 < SLOTS:
                    s = t - 1
                    h1t = h1_pool.tile([128, 2 * L], bf16, name=f"h1_{s}",
                                       tag="h1")
                    h1_t[s] = h1t
                    nc.vector.tensor_scalar_max(h1t[:], ps1_t.pop(s)[:], 0.0)
                # ---- stage 3: MM2 (two concurrent M=64 matmuls per pair)
                if 0 <= t - 2 < SLOTS:
                    s = t - 2
                    h1t = h1_t[s]
                    ps2 = psB_pool.tile([128, 2 * L], f32, name=f"ps2_{s}",
                                        tag="psB")
                    ps2_t[s] = ps2
                    for u in range(2):
                        sl = slice(u * L, (u + 1) * L)
                        nc.tensor.matmul(
                            out=ps2[0:64, sl],
                            lhsT=w2t[0:64, :],
                            rhs=h1t[0:64, sl],
                            start=True, stop=True,
                            tile_position=(0, 0),
                        )
                        nc.tensor.matmul(
                            out=ps2[64:128, sl],
                            lhsT=w2t[64:128, :],
                            rhs=h1t[64:128, sl],
                            start=True, stop=True,
                            tile_position=(64, 64),
                        )
                    del h1_t[s]
                # ---- stage 4: relu2 on ACT (sometimes DVE for balance)
                if 0 <= t - 3 < SLOTS:
                    s = t - 3
                    h2t = h2_pool.tile([128, 2 * L], bf16, name=f"h2_{s}",
                                       tag="h2")
                    h2_t[s] = h2t
                    if s % REBAL == REBAL - 1:
                        nc.vector.tensor_scalar_max(h2t[:], ps2_t.pop(s)[:],
                                                    0.0)
                    else:
                        nc.scalar.activation(h2t[:], ps2_t.pop(s)[:], Act.Relu)
    nc.finalize()
    return nc


def _get_program():
    if "nc" not in _cache:
        _cache["nc"] = _build_program()
    return _cache["nc"]


def _pack_inputs(pos, normal, emb, W1):
    """Host-side: hash + table lookup + bake transposed bf16 tiles.

    pair p (q = p%8 in macro): partition base 64*(q%2) + 32*e, col slot
    q//2; chunks 2p (e=0) and 2p+1 (e=1)."""
    idx = _hash_idx(pos)
    x19 = np.empty((N, 19), np.float32)
    x19[:, :FEAT] = emb[idx]
    x19[:, FEAT:] = normal
    xv = x19.astype(ml_dtypes.bfloat16)
    r = xv.reshape(NC, MACROS, 8, 2, L, 19)     # [k, m, q, e, j, f]
    xt = np.zeros((NC, MACROS, 2, 64, 4, L), ml_dtypes.bfloat16)
    for q in range(8):
        a, c = q % 2, q // 2
        for e in range(2):
            xt[:, :, a, 19 * e:19 * e + 19, c, :] = (
                r[:, :, q, e].transpose(0, 1, 3, 2))
    return xt.reshape(NC, MACROS, 128, 2048)


def _bake_weights(W1, W2, W3):
    w1 = np.zeros((128, 128), ml_dtypes.bfloat16)
    for base in (0, 64):
        w1[base + 0:base + 19, 0:64] = W1.astype(ml_dtypes.bfloat16)
        w1[base + 19:base + 38, 64:128] = W1.astype(ml_dtypes.bfloat16)
    w2 = np.empty((128, H), ml_dtypes.bfloat16)
    w2[0:64] = W2.astype(ml_dtypes.bfloat16)
    w2[64:128] = W2.astype(ml_dtypes.bfloat16)
    w3 = np.zeros((128, 32), ml_dtypes.bfloat16)
    w3[0:64, 0:3] = W3.astype(ml_dtypes.bfloat16)
    w3[64:128, 3:6] = W3.astype(ml_dtypes.bfloat16)
    return w1, w2, w3


def kernel(pos, normal, emb, W1, b1, W2, b2, W3, b3):
    from concourse.bass_utils import run_bass_kernel_spmd

    assert not np.any(b1) and not np.any(b2) and not np.any(b3), (
        "nonzero biases not supported by this kernel build")

    nc = _get_program()
    xt = _pack_inputs(np.asarray(pos), np.asarray(normal), np.asarray(emb),
                      np.asarray(W1))
    w1, w2, w3 = _bake_weights(np.asarray(W1), np.asarray(W2), np.asarray(W3))
    in_maps = [
        {"xt": xt[k], "w1": w1, "w2": w2, "w3": w3}
        for k in range(NC)
    ]
    res = run_bass_kernel_spmd(nc, in_maps, core_ids=list(range(NC)))
    return _unpack(res)


def _unpack(res):
    od = np.stack([np.asarray(res.results[k]["out"]) for k in range(NC)])
    # od: [core, g, 32r+o, h*L+j2]; pair p = 8g+4h+r; row=(2p+e)*L+j2
    od = od.reshape(NC, GROUPS, 4, 32, 2, L)[:, :, :, 0:6]
    od = od.reshape(NC, GROUPS, 4, 2, 3, 2, L)   # [k, g, r, e, c, h, j2]
    od = np.transpose(od, (0, 1, 5, 2, 3, 6, 4))  # [k, g, h, r, e, j2, c]
    return np.ascontiguousarray(od.reshape(N, 3).astype(np.float32))
